# revision 7
# baseline (speedup 1.0000x reference)
"""AGConv Trainium2 kernel: 8-core SPMD over the num_points axis.

Reference computation (per full tensors):
  h   = einsum('of,bfnk->bonk', W0, y)             # conv0
  h   = leaky(bn0(h))                              # BN training mode + LeakyReLU(0.2)
  w   = einsum('po,bonk->bpnk', W1, h)             # conv1 -> per-edge [C_out, C_in] mats
  out = einsum('bnkoi,bink->bonk', w_reshaped, x)  # per-edge tiny matmul
  out = leaky(bn1(out))

Key algebra used here: the per-edge tiny matmul folds into one dense matmul,
  out[o,p] = sum_{i,c} A2[i*64+c, o] * Z[i*64+c, p]
with A2[i*64+c, o] = W1[o*6+i, c] and Z[i*64+c, p] = x[i,p] * hhat[c,p].
Z is built from hhat duplicated across partition halves (via a doubled W0^T
stationary) times x rows broadcast across 64 partitions (via tiny indicator
matmuls). BN stats are computed with bn_stats/bn_aggr and all-reduced across
the 8 cores (sum / sum-of-squares payloads).

Sharding: num_points N=2048 is split 8 ways (256 points per core); weights
and BN params are replicated; output gathered by concatenation over N.
"""

import numpy as np

import concourse.bass as bass
import concourse.bacc as bacc
import concourse.mybir as mybir
import concourse.tile as tile
from concourse.bass_utils import run_bass_kernel_spmd

EPS = 1e-5
SLOPE = 0.2

B, C_IN, N, K = 4, 6, 2048, 40
C_FEAT, C_OUT = 12, 64
NCORES = 8
N_LOC = N // NCORES          # 256 points per core
M_B = N_LOC * K              # 10240 positions per batch per core
M = B * M_B                  # 40960 positions per core
FCH = 512                    # free-dim chunk (1 PSUM bank of f32)
CH_PER_B = M_B // FCH        # 20 chunks per batch
NCH = B * CH_PER_B           # 80 chunks per core
HB = 2                       # half-batches per batch for staging loads
CH_PER_HB = CH_PER_B // HB   # 10
M_HB = M_B // HB             # 5120

M_TOT = float(NCORES * M)            # 327680 positions globally
M1_HALF = float(NCORES * M // 2)     # per-partition sample count for stats1

F32 = mybir.dt.float32
BF16 = mybir.dt.bfloat16

TRACE = False
LAST_EXEC_NS = None
_CACHE = {}


def _build():
    nc = bacc.Bacc(
        "TRN2", target_bir_lowering=False, debug=False, num_devices=NCORES,
    )

    y_ext = nc.declare_dram_parameter("y", [B, C_FEAT, N_LOC, K], F32, isOutput=False)
    x_ext = nc.declare_dram_parameter("x", [B, C_IN, N_LOC, K], F32, isOutput=False)
    w0t2_ext = nc.declare_dram_parameter("w0t2", [C_FEAT, 128], F32, isOutput=False)
    a2_ext = nc.declare_dram_parameter("a2", [128, 192], F32, isOutput=False)
    ind_ext = nc.declare_dram_parameter("ind", [C_IN, 384], F32, isOutput=False)
    g0_ext = nc.declare_dram_parameter("g0b0", [128, 2], F32, isOutput=False)
    g1_ext = nc.declare_dram_parameter("g1b1", [128, 2], F32, isOutput=False)
    out_ext = nc.declare_dram_parameter("out", [B, C_OUT, N_LOC, K], F32, isOutput=True)

    y_v = y_ext.ap().rearrange("b c n k -> b c (n k)")
    x_v = x_ext.ap().rearrange("b c n k -> b c (n k)")
    out_v = out_ext.ap().rearrange("b c n k -> b c (n k)")

    with tile.TileContext(nc) as tc:
        with (
            tc.tile_pool(name="const", bufs=1) as cpool,
            tc.tile_pool(name="big", bufs=1) as bpool,
            tc.tile_pool(name="yld", bufs=2) as ypool,
            tc.tile_pool(name="xld", bufs=2) as xpool,
            tc.tile_pool(name="work", bufs=3) as wpool,
            tc.tile_pool(name="stat", bufs=1) as spool,
            tc.tile_pool(name="pa", bufs=2, space="PSUM") as pa_pool,
            tc.tile_pool(name="px", bufs=1, space="PSUM") as px_pool,
            tc.tile_pool(name="pc", bufs=2, space="PSUM") as pc_pool,
            tc.tile_pool(name="dram", bufs=1, space="DRAM") as dpool,
        ):
            # ---- constants ----
            w0t2 = cpool.tile([C_FEAT, 128], F32, tag="w0t2")
            a2w = cpool.tile([128, 192], F32, tag="a2w")
            ind = cpool.tile([C_IN, 384], F32, tag="ind")
            g0b0 = cpool.tile([128, 2], F32, tag="g0b0")
            g1b1 = cpool.tile([128, 2], F32, tag="g1b1")
            nc.sync.dma_start(out=w0t2[:], in_=w0t2_ext.ap())
            nc.sync.dma_start(out=a2w[:], in_=a2_ext.ap())
            nc.sync.dma_start(out=ind[:], in_=ind_ext.ap())
            nc.sync.dma_start(out=g0b0[:], in_=g0_ext.ap())
            nc.sync.dma_start(out=g1b1[:], in_=g1_ext.ap())

            # ---- persistent buffers ----
            op_all = bpool.tile([128, M // 2], BF16, tag="op_all")  # 40KB/part
            st0 = spool.tile([128, 6 * NCH], F32, tag="st0")
            st1 = spool.tile([128, 6 * (NCH // 2)], F32, tag="st1")

            # =========== Phase A: h = W0 @ y (doubled), stats0 ===========
            for b in range(B):
                for hb in range(HB):
                    y_hb = ypool.tile([C_FEAT, M_HB], F32, tag="y_hb")
                    s = hb * M_HB
                    nc.sync.dma_start(out=y_hb[:], in_=y_v[b, :, s:s + M_HB])
                    for t in range(CH_PER_HB):
                        idx = (b * HB + hb) * CH_PER_HB + t
                        pA = pa_pool.tile([128, FCH], F32, tag="pA")
                        nc.tensor.matmul(
                            pA[:], w0t2[:], y_hb[:, t * FCH:(t + 1) * FCH],
                            start=True, stop=True,
                        )
                        nc.vector.bn_stats(st0[:, idx * 6:(idx + 1) * 6], pA[:])

            # =========== All-reduce stats0 -> a0/b0 scale-bias ===========
            agg0 = spool.tile([128, 2], F32, tag="agg0")
            pay0 = spool.tile([128, 2], F32, tag="pay0")
            nc.vector.bn_aggr(agg0[:], st0[:])
            # payload: [sum, sumsq] = [mean*M, (var+mean^2)*M]
            m2a = spool.tile([128, 1], F32, tag="m2a")
            nc.vector.tensor_mul(m2a[:], agg0[:, 0:1], agg0[:, 0:1])
            nc.vector.tensor_add(pay0[:, 1:2], agg0[:, 1:2], m2a[:])
            nc.vector.tensor_copy(pay0[:, 0:1], agg0[:, 0:1])
            nc.vector.tensor_scalar_mul(pay0[:], pay0[:], float(M))

            cc0_in = dpool.tile([128, 2], F32, tag="cc0i")
            cc0_out = dpool.tile([128, 2], F32, tag="cc0o")
            nc.sync.dma_start(out=cc0_in[:], in_=pay0[:])
            nc.gpsimd.collective_compute(
                "AllReduce", mybir.AluOpType.add,
                replica_groups=[list(range(NCORES))],
                ins=[cc0_in.opt()], outs=[cc0_out.opt()],
            )
            gs0 = spool.tile([128, 2], F32, tag="gs0")
            nc.sync.dma_start(out=gs0[:], in_=cc0_out[:])

            ab0 = spool.tile([128, 2], F32, tag="ab0")  # col0=a, col1=b
            mean0 = spool.tile([128, 1], F32, tag="mean0")
            var0 = spool.tile([128, 1], F32, tag="var0")
            sd0 = spool.tile([128, 1], F32, tag="sd0")
            t0 = spool.tile([128, 1], F32, tag="t0")
            nc.vector.tensor_scalar_mul(gs0[:], gs0[:], 1.0 / M_TOT)
            nc.vector.tensor_copy(mean0[:], gs0[:, 0:1])
            nc.vector.tensor_mul(t0[:], mean0[:], mean0[:])
            nc.vector.tensor_sub(var0[:], gs0[:, 1:2], t0[:])
            nc.vector.tensor_scalar_add(var0[:], var0[:], EPS)
            nc.scalar.activation(sd0[:], var0[:], mybir.ActivationFunctionType.Sqrt)
            nc.vector.reciprocal(sd0[:], sd0[:])
            nc.vector.tensor_mul(ab0[:, 0:1], sd0[:], g0b0[:, 0:1])
            nc.vector.tensor_mul(t0[:], ab0[:, 0:1], mean0[:])
            nc.vector.tensor_sub(ab0[:, 1:2], g0b0[:, 1:2], t0[:])

            # =========== Phase B: hh=lrelu(a*h+b); Z; out_pre; stats1 ====
            for b in range(B):
                for hb in range(HB):
                    x_hb = xpool.tile([C_IN, M_HB], F32, tag="x_hb")
                    y_hb = ypool.tile([C_FEAT, M_HB], F32, tag="y_hb")
                    s = hb * M_HB
                    nc.sync.dma_start(out=x_hb[:], in_=x_v[b, :, s:s + M_HB])
                    nc.sync.dma_start(out=y_hb[:], in_=y_v[b, :, s:s + M_HB])
                    for t in range(CH_PER_HB):
                        idx = (b * HB + hb) * CH_PER_HB + t
                        pB = pa_pool.tile([128, FCH], F32, tag="pA")
                        nc.tensor.matmul(
                            pB[:], w0t2[:], y_hb[:, t * FCH:(t + 1) * FCH],
                            start=True, stop=True,
                        )
                        hh2 = wpool.tile([128, FCH], F32, tag="hh2")
                        nc.scalar.activation(
                            hh2[:], pB[:],
                            mybir.ActivationFunctionType.Prelu,
                            bias=ab0[:, 1:2], scale=ab0[:, 0:1], alpha=SLOPE,
                        )
                        if idx % 2 == 0:
                            pC = pc_pool.tile([128, FCH], F32, tag="pC")
                        half = pC[0:64, :] if idx % 2 == 0 else pC[64:128, :]
                        for j in range(3):
                            pX = px_pool.tile([128, FCH], F32, tag=f"px{j}")
                            nc.tensor.matmul(
                                pX[:], ind[:, 128 * j:128 * (j + 1)],
                                x_hb[:, t * FCH:(t + 1) * FCH],
                                start=True, stop=True,
                            )
                            Z = wpool.tile([128, FCH], F32, tag=f"z{j}")
                            nc.vector.tensor_mul(Z[:], hh2[:], pX[:])
                            nc.tensor.matmul(
                                half, a2w[:, 64 * j:64 * (j + 1)], Z[:],
                                start=(j == 0), stop=(j == 2),
                            )
                        if idx % 2 == 1:
                            pr = idx // 2
                            nc.vector.bn_stats(st1[:, pr * 6:(pr + 1) * 6], pC[:])
                            nc.vector.tensor_copy(
                                op_all[:, pr * FCH:(pr + 1) * FCH], pC[:])

            # =========== All-reduce stats1 -> a1/b1 ===========
            agg1 = spool.tile([128, 2], F32, tag="agg1")
            pay1 = spool.tile([128, 2], F32, tag="pay1")
            nc.vector.bn_aggr(agg1[:], st1[:])
            m2b = spool.tile([128, 1], F32, tag="m2b")
            nc.vector.tensor_mul(m2b[:], agg1[:, 0:1], agg1[:, 0:1])
            nc.vector.tensor_add(pay1[:, 1:2], agg1[:, 1:2], m2b[:])
            nc.vector.tensor_copy(pay1[:, 0:1], agg1[:, 0:1])
            nc.vector.tensor_scalar_mul(pay1[:], pay1[:], float(M // 2))

            cc1_in = dpool.tile([128, 2], F32, tag="cc1i")
            cc1_out = dpool.tile([128, 2], F32, tag="cc1o")
            nc.sync.dma_start(out=cc1_in[:], in_=pay1[:])
            nc.gpsimd.collective_compute(
                "AllReduce", mybir.AluOpType.add,
                replica_groups=[list(range(NCORES))],
                ins=[cc1_in.opt()], outs=[cc1_out.opt()],
            )
            gs1 = spool.tile([128, 2], F32, tag="gs1")
            nc.sync.dma_start(out=gs1[:], in_=cc1_out[:])

            # combine partition halves (each half saw different chunks)
            tmph = spool.tile([64, 2], F32, tag="tmph")
            tot1 = spool.tile([64, 2], F32, tag="tot1")
            nc.sync.dma_start(out=tmph[:], in_=gs1[64:128, :])
            nc.vector.tensor_add(tot1[:], gs1[0:64, :], tmph[:])

            ab1 = spool.tile([128, 2], F32, tag="ab1")
            mean1 = spool.tile([64, 1], F32, tag="mean1")
            var1 = spool.tile([64, 1], F32, tag="var1")
            sd1 = spool.tile([64, 1], F32, tag="sd1")
            t1 = spool.tile([64, 1], F32, tag="t1")
            nc.vector.tensor_scalar_mul(tot1[:], tot1[:], 1.0 / M_TOT)
            nc.vector.tensor_copy(mean1[:], tot1[:, 0:1])
            nc.vector.tensor_mul(t1[:], mean1[:], mean1[:])
            nc.vector.tensor_sub(var1[:], tot1[:, 1:2], t1[:])
            nc.vector.tensor_scalar_add(var1[:], var1[:], EPS)
            nc.scalar.activation(sd1[:], var1[:], mybir.ActivationFunctionType.Sqrt)
            nc.vector.reciprocal(sd1[:], sd1[:])
            nc.vector.tensor_mul(ab1[0:64, 0:1], sd1[:], g1b1[0:64, 0:1])
            nc.vector.tensor_mul(t1[:], ab1[0:64, 0:1], mean1[:])
            nc.vector.tensor_sub(ab1[0:64, 1:2], g1b1[0:64, 1:2], t1[:])
            nc.sync.dma_start(out=ab1[64:128, :], in_=ab1[0:64, :])

            # =========== Phase C: out = lrelu(a1*out_pre+b1) -> DRAM =====
            for pr in range(NCH // 2):
                outf = wpool.tile([128, FCH], F32, tag="outf")
                nc.scalar.activation(
                    outf[:], op_all[:, pr * FCH:(pr + 1) * FCH],
                    mybir.ActivationFunctionType.Prelu,
                    bias=ab1[:, 1:2], scale=ab1[:, 0:1], alpha=SLOPE,
                )
                for half in range(2):
                    c = 2 * pr + half
                    b = c // CH_PER_B
                    tb = c % CH_PER_B
                    nc.sync.dma_start(
                        out=out_v[b, :, tb * FCH:(tb + 1) * FCH],
                        in_=outf[64 * half:64 * (half + 1), :],
                    )
    nc.finalize()
    return nc


def _prep_weights(W0, W1, gamma0, beta0, gamma1, beta1):
    W0 = np.asarray(W0, np.float32)
    W1 = np.asarray(W1, np.float32)
    w0t2 = np.concatenate([W0.T, W0.T], axis=1)  # [12, 128]
    # A2[i*64+c, o] = W1[o*6+i, c]
    A2 = W1.reshape(C_OUT, C_IN, C_OUT).transpose(1, 2, 0).reshape(384, C_OUT)
    a2p = np.concatenate([A2[0:128], A2[128:256], A2[256:384]], axis=1)  # [128,192]
    ind = np.zeros((C_IN, 384), np.float32)
    for j in range(3):
        for p in range(128):
            ind[2 * j + p // 64, 128 * j + p] = 1.0
    g0b0 = np.stack([np.tile(np.asarray(gamma0, np.float32), 2),
                     np.tile(np.asarray(beta0, np.float32), 2)], axis=1)
    g1b1 = np.stack([np.tile(np.asarray(gamma1, np.float32), 2),
                     np.tile(np.asarray(beta1, np.float32), 2)], axis=1)
    return (np.ascontiguousarray(w0t2, np.float32),
            np.ascontiguousarray(a2p, np.float32),
            np.ascontiguousarray(ind, np.float32),
            np.ascontiguousarray(g0b0, np.float32),
            np.ascontiguousarray(g1b1, np.float32))


def kernel(x, y, W0, gamma0, beta0, W1, gamma1, beta1):
    global LAST_EXEC_NS
    x = np.asarray(x, np.float32)
    y = np.asarray(y, np.float32)
    w0t2, a2p, ind, g0b0, g1b1 = _prep_weights(W0, W1, gamma0, beta0, gamma1, beta1)

    if "nc" not in _CACHE:
        _CACHE["nc"] = _build()
    nc = _CACHE["nc"]

    in_maps = []
    for core in range(NCORES):
        n0 = core * N_LOC
        in_maps.append({
            "y": np.ascontiguousarray(y[:, :, n0:n0 + N_LOC, :]),
            "x": np.ascontiguousarray(x[:, :, n0:n0 + N_LOC, :]),
            "w0t2": w0t2, "a2": a2p, "ind": ind,
            "g0b0": g0b0, "g1b1": g1b1,
        })
    res = run_bass_kernel_spmd(nc, in_maps, list(range(NCORES)), trace=TRACE)
    LAST_EXEC_NS = res.exec_time_ns
    out = np.concatenate([r["out"] for r in res.results], axis=2)
    return out


# revision 9
# speedup vs baseline: 2.0058x; 2.0058x over previous
"""AGConv Trainium2 kernel: 8-core SPMD over the num_points axis.

Reference computation (per full tensors):
  h   = einsum('of,bfnk->bonk', W0, y)             # conv0
  h   = leaky(bn0(h))                              # BN training mode + LeakyReLU(0.2)
  w   = einsum('po,bonk->bpnk', W1, h)             # conv1 -> per-edge [C_out, C_in] mats
  out = einsum('bnkoi,bink->bonk', w_reshaped, x)  # per-edge tiny matmul
  out = leaky(bn1(out))

Key algebra used here: the per-edge tiny matmul folds into one dense matmul,
  out[o,p] = sum_{i,c} A2[i*64+c, o] * Z[i*64+c, p]
with A2[i*64+c, o] = W1[o*6+i, c] and Z[i*64+c, p] = x[i,p] * hhat[c,p].
Z is built from hhat duplicated across partition halves (via a doubled W0^T
stationary) times x rows broadcast across 64 partitions (via tiny indicator
matmuls). BN stats are computed with bn_stats/bn_aggr and all-reduced across
the 8 cores (sum / sum-of-squares payloads).

Sharding: num_points N=2048 is split 8 ways (256 points per core); weights
and BN params are replicated; output gathered by concatenation over N.
"""

import numpy as np

import concourse.bass as bass
import concourse.bacc as bacc
import concourse.mybir as mybir
import concourse.tile as tile
from concourse.bass_utils import run_bass_kernel_spmd

EPS = 1e-5
SLOPE = 0.2

B, C_IN, N, K = 4, 6, 2048, 40
C_FEAT, C_OUT = 12, 64
NCORES = 8
N_LOC = N // NCORES          # 256 points per core
M_B = N_LOC * K              # 10240 positions per batch per core
M = B * M_B                  # 40960 positions per core
FCH = 512                    # free-dim chunk (1 PSUM bank of f32)
CH_PER_B = M_B // FCH        # 20 chunks per batch
NCH = B * CH_PER_B           # 80 chunks per core
HB = 2                       # half-batches per batch for staging loads
CH_PER_HB = CH_PER_B // HB   # 10
M_HB = M_B // HB             # 5120

M_TOT = float(NCORES * M)            # 327680 positions globally
M1_HALF = float(NCORES * M // 2)     # per-partition sample count for stats1

F32 = mybir.dt.float32
BF16 = mybir.dt.bfloat16

TRACE = False
LAST_EXEC_NS = None
_CACHE = {}


def _build():
    nc = bacc.Bacc(
        "TRN2", target_bir_lowering=False, debug=False, num_devices=NCORES,
    )

    y_ext = nc.declare_dram_parameter("y", [B, C_FEAT, N_LOC, K], F32, isOutput=False)
    x_ext = nc.declare_dram_parameter("x", [B, C_IN, N_LOC, K], F32, isOutput=False)
    w0t2_ext = nc.declare_dram_parameter("w0t2", [C_FEAT, 128], F32, isOutput=False)
    a2_ext = nc.declare_dram_parameter("a2", [128, 192], F32, isOutput=False)
    ind_ext = nc.declare_dram_parameter("ind", [C_IN, 384], F32, isOutput=False)
    g0_ext = nc.declare_dram_parameter("g0b0", [128, 2], F32, isOutput=False)
    g1_ext = nc.declare_dram_parameter("g1b1", [128, 2], F32, isOutput=False)
    out_ext = nc.declare_dram_parameter("out", [B, C_OUT, N_LOC, K], F32, isOutput=True)

    y_v = y_ext.ap().rearrange("b c n k -> b c (n k)")
    x_v = x_ext.ap().rearrange("b c n k -> b c (n k)")
    out_v = out_ext.ap().rearrange("b c n k -> b c (n k)")

    with tile.TileContext(nc) as tc:
        with (
            tc.tile_pool(name="const", bufs=1) as cpool,
            tc.tile_pool(name="big", bufs=1) as bpool,
            tc.tile_pool(name="yld", bufs=2) as ypool,
            tc.tile_pool(name="xld", bufs=2) as xpool,
            tc.tile_pool(name="work", bufs=3) as wpool,
            tc.tile_pool(name="stat", bufs=1) as spool,
            tc.tile_pool(name="pa", bufs=2, space="PSUM") as pa_pool,
            tc.tile_pool(name="px", bufs=1, space="PSUM") as px_pool,
            tc.tile_pool(name="pc", bufs=2, space="PSUM") as pc_pool,
            tc.tile_pool(name="dram", bufs=1, space="DRAM") as dpool,
        ):
            # ---- constants ----
            w0t2 = cpool.tile([C_FEAT, 128], BF16, tag="w0t2")
            a2w = cpool.tile([128, 192], BF16, tag="a2w")
            ind = cpool.tile([C_IN, 384], BF16, tag="ind")
            g0b0 = cpool.tile([128, 2], F32, tag="g0b0")
            g1b1 = cpool.tile([128, 2], F32, tag="g1b1")
            nc.gpsimd.dma_start(out=w0t2[:], in_=w0t2_ext.ap())
            nc.gpsimd.dma_start(out=a2w[:], in_=a2_ext.ap())
            nc.gpsimd.dma_start(out=ind[:], in_=ind_ext.ap())
            nc.sync.dma_start(out=g0b0[:], in_=g0_ext.ap())
            nc.sync.dma_start(out=g1b1[:], in_=g1_ext.ap())

            # ---- persistent buffers ----
            op_all = bpool.tile([128, M // 2], BF16, tag="op_all")  # 40KB/part
            st0 = spool.tile([128, 6 * NCH], F32, tag="st0")
            st1 = spool.tile([128, 6 * (NCH // 2)], F32, tag="st1")

            # =========== Phase A: h = W0 @ y (doubled), stats0 ===========
            for b in range(B):
                for hb in range(HB):
                    y_hb = ypool.tile([C_FEAT, M_HB], BF16, tag="y_hb")
                    s = hb * M_HB
                    nc.gpsimd.dma_start(out=y_hb[:], in_=y_v[b, :, s:s + M_HB])
                    for t in range(CH_PER_HB):
                        idx = (b * HB + hb) * CH_PER_HB + t
                        pA = pa_pool.tile([128, FCH], F32, tag="pA")
                        nc.tensor.matmul(
                            pA[:], w0t2[:], y_hb[:, t * FCH:(t + 1) * FCH],
                            start=True, stop=True,
                        )
                        nc.vector.bn_stats(st0[:, idx * 6:(idx + 1) * 6], pA[:])

            # =========== All-reduce stats0 -> a0/b0 scale-bias ===========
            agg0 = spool.tile([128, 2], F32, tag="agg0")
            pay0 = spool.tile([128, 2], F32, tag="pay0")
            nc.vector.bn_aggr(agg0[:], st0[:])
            # payload: [sum, sumsq] = [mean*M, (var+mean^2)*M]
            m2a = spool.tile([128, 1], F32, tag="m2a")
            nc.vector.tensor_mul(m2a[:], agg0[:, 0:1], agg0[:, 0:1])
            nc.vector.tensor_add(pay0[:, 1:2], agg0[:, 1:2], m2a[:])
            nc.vector.tensor_copy(pay0[:, 0:1], agg0[:, 0:1])
            nc.vector.tensor_scalar_mul(pay0[:], pay0[:], float(M))

            cc0_in = dpool.tile([128, 2], F32, tag="cc0i")
            cc0_out = dpool.tile([128, 2], F32, tag="cc0o")
            nc.sync.dma_start(out=cc0_in[:], in_=pay0[:])
            nc.gpsimd.collective_compute(
                "AllReduce", mybir.AluOpType.add,
                replica_groups=[list(range(NCORES))],
                ins=[cc0_in.opt()], outs=[cc0_out.opt()],
            )
            gs0 = spool.tile([128, 2], F32, tag="gs0")
            nc.sync.dma_start(out=gs0[:], in_=cc0_out[:])

            ab0 = spool.tile([128, 2], F32, tag="ab0")  # col0=a, col1=b
            mean0 = spool.tile([128, 1], F32, tag="mean0")
            var0 = spool.tile([128, 1], F32, tag="var0")
            sd0 = spool.tile([128, 1], F32, tag="sd0")
            t0 = spool.tile([128, 1], F32, tag="t0")
            nc.vector.tensor_scalar_mul(gs0[:], gs0[:], 1.0 / M_TOT)
            nc.vector.tensor_copy(mean0[:], gs0[:, 0:1])
            nc.vector.tensor_mul(t0[:], mean0[:], mean0[:])
            nc.vector.tensor_sub(var0[:], gs0[:, 1:2], t0[:])
            nc.vector.tensor_scalar_add(var0[:], var0[:], EPS)
            nc.scalar.activation(sd0[:], var0[:], mybir.ActivationFunctionType.Sqrt)
            nc.vector.reciprocal(sd0[:], sd0[:])
            nc.vector.tensor_mul(ab0[:, 0:1], sd0[:], g0b0[:, 0:1])
            nc.vector.tensor_mul(t0[:], ab0[:, 0:1], mean0[:])
            nc.vector.tensor_sub(ab0[:, 1:2], g0b0[:, 1:2], t0[:])

            # =========== Phase B: hh=lrelu(a*h+b); Z; out_pre; stats1 ====
            for b in range(B):
                for hb in range(HB):
                    x_hb = xpool.tile([C_IN, M_HB], BF16, tag="x_hb")
                    y_hb = ypool.tile([C_FEAT, M_HB], BF16, tag="y_hb")
                    s = hb * M_HB
                    nc.gpsimd.dma_start(out=x_hb[:], in_=x_v[b, :, s:s + M_HB])
                    nc.gpsimd.dma_start(out=y_hb[:], in_=y_v[b, :, s:s + M_HB])
                    for t in range(CH_PER_HB):
                        idx = (b * HB + hb) * CH_PER_HB + t
                        pB = pa_pool.tile([128, FCH], F32, tag="pA")
                        nc.tensor.matmul(
                            pB[:], w0t2[:], y_hb[:, t * FCH:(t + 1) * FCH],
                            start=True, stop=True,
                        )
                        hh2 = wpool.tile([128, FCH], BF16, tag="hh2")
                        nc.scalar.activation(
                            hh2[:], pB[:],
                            mybir.ActivationFunctionType.Prelu,
                            bias=ab0[:, 1:2], scale=ab0[:, 0:1], alpha=SLOPE,
                        )
                        if idx % 2 == 0:
                            pC = pc_pool.tile([128, FCH], F32, tag="pC")
                        half = pC[0:64, :] if idx % 2 == 0 else pC[64:128, :]
                        for j in range(3):
                            pX = px_pool.tile([128, FCH], F32, tag=f"px{j}")
                            nc.tensor.matmul(
                                pX[:], ind[:, 128 * j:128 * (j + 1)],
                                x_hb[:, t * FCH:(t + 1) * FCH],
                                start=True, stop=True,
                            )
                            xb = wpool.tile([128, FCH], BF16, tag=f"xb{j}")
                            nc.scalar.copy(out=xb[:], in_=pX[:])
                            Z = wpool.tile([128, FCH], BF16, tag=f"z{j}")
                            nc.vector.tensor_mul(Z[:], hh2[:], xb[:])
                            nc.tensor.matmul(
                                half, a2w[:, 64 * j:64 * (j + 1)], Z[:],
                                start=(j == 0), stop=(j == 2),
                            )
                        if idx % 2 == 1:
                            pr = idx // 2
                            nc.vector.bn_stats(st1[:, pr * 6:(pr + 1) * 6], pC[:])
                            nc.vector.tensor_copy(
                                op_all[:, pr * FCH:(pr + 1) * FCH], pC[:])

            # =========== All-reduce stats1 -> a1/b1 ===========
            agg1 = spool.tile([128, 2], F32, tag="agg1")
            pay1 = spool.tile([128, 2], F32, tag="pay1")
            nc.vector.bn_aggr(agg1[:], st1[:])
            m2b = spool.tile([128, 1], F32, tag="m2b")
            nc.vector.tensor_mul(m2b[:], agg1[:, 0:1], agg1[:, 0:1])
            nc.vector.tensor_add(pay1[:, 1:2], agg1[:, 1:2], m2b[:])
            nc.vector.tensor_copy(pay1[:, 0:1], agg1[:, 0:1])
            nc.vector.tensor_scalar_mul(pay1[:], pay1[:], float(M // 2))

            cc1_in = dpool.tile([128, 2], F32, tag="cc1i")
            cc1_out = dpool.tile([128, 2], F32, tag="cc1o")
            nc.sync.dma_start(out=cc1_in[:], in_=pay1[:])
            nc.gpsimd.collective_compute(
                "AllReduce", mybir.AluOpType.add,
                replica_groups=[list(range(NCORES))],
                ins=[cc1_in.opt()], outs=[cc1_out.opt()],
            )
            gs1 = spool.tile([128, 2], F32, tag="gs1")
            nc.sync.dma_start(out=gs1[:], in_=cc1_out[:])

            # combine partition halves (each half saw different chunks)
            tmph = spool.tile([64, 2], F32, tag="tmph")
            tot1 = spool.tile([64, 2], F32, tag="tot1")
            nc.sync.dma_start(out=tmph[:], in_=gs1[64:128, :])
            nc.vector.tensor_add(tot1[:], gs1[0:64, :], tmph[:])

            ab1 = spool.tile([128, 2], F32, tag="ab1")
            mean1 = spool.tile([64, 1], F32, tag="mean1")
            var1 = spool.tile([64, 1], F32, tag="var1")
            sd1 = spool.tile([64, 1], F32, tag="sd1")
            t1 = spool.tile([64, 1], F32, tag="t1")
            nc.vector.tensor_scalar_mul(tot1[:], tot1[:], 1.0 / M_TOT)
            nc.vector.tensor_copy(mean1[:], tot1[:, 0:1])
            nc.vector.tensor_mul(t1[:], mean1[:], mean1[:])
            nc.vector.tensor_sub(var1[:], tot1[:, 1:2], t1[:])
            nc.vector.tensor_scalar_add(var1[:], var1[:], EPS)
            nc.scalar.activation(sd1[:], var1[:], mybir.ActivationFunctionType.Sqrt)
            nc.vector.reciprocal(sd1[:], sd1[:])
            nc.vector.tensor_mul(ab1[0:64, 0:1], sd1[:], g1b1[0:64, 0:1])
            nc.vector.tensor_mul(t1[:], ab1[0:64, 0:1], mean1[:])
            nc.vector.tensor_sub(ab1[0:64, 1:2], g1b1[0:64, 1:2], t1[:])
            nc.sync.dma_start(out=ab1[64:128, :], in_=ab1[0:64, :])

            # =========== Phase C: out = lrelu(a1*out_pre+b1) -> DRAM =====
            for pr in range(NCH // 2):
                outf = wpool.tile([128, FCH], F32, tag="outf")
                nc.scalar.activation(
                    outf[:], op_all[:, pr * FCH:(pr + 1) * FCH],
                    mybir.ActivationFunctionType.Prelu,
                    bias=ab1[:, 1:2], scale=ab1[:, 0:1], alpha=SLOPE,
                )
                for half in range(2):
                    c = 2 * pr + half
                    b = c // CH_PER_B
                    tb = c % CH_PER_B
                    nc.sync.dma_start(
                        out=out_v[b, :, tb * FCH:(tb + 1) * FCH],
                        in_=outf[64 * half:64 * (half + 1), :],
                    )
    nc.finalize()
    return nc


def _prep_weights(W0, W1, gamma0, beta0, gamma1, beta1):
    W0 = np.asarray(W0, np.float32)
    W1 = np.asarray(W1, np.float32)
    w0t2 = np.concatenate([W0.T, W0.T], axis=1)  # [12, 128]
    # A2[i*64+c, o] = W1[o*6+i, c]
    A2 = W1.reshape(C_OUT, C_IN, C_OUT).transpose(1, 2, 0).reshape(384, C_OUT)
    a2p = np.concatenate([A2[0:128], A2[128:256], A2[256:384]], axis=1)  # [128,192]
    ind = np.zeros((C_IN, 384), np.float32)
    for j in range(3):
        for p in range(128):
            ind[2 * j + p // 64, 128 * j + p] = 1.0
    g0b0 = np.stack([np.tile(np.asarray(gamma0, np.float32), 2),
                     np.tile(np.asarray(beta0, np.float32), 2)], axis=1)
    g1b1 = np.stack([np.tile(np.asarray(gamma1, np.float32), 2),
                     np.tile(np.asarray(beta1, np.float32), 2)], axis=1)
    return (np.ascontiguousarray(w0t2, np.float32),
            np.ascontiguousarray(a2p, np.float32),
            np.ascontiguousarray(ind, np.float32),
            np.ascontiguousarray(g0b0, np.float32),
            np.ascontiguousarray(g1b1, np.float32))


def kernel(x, y, W0, gamma0, beta0, W1, gamma1, beta1):
    global LAST_EXEC_NS
    x = np.asarray(x, np.float32)
    y = np.asarray(y, np.float32)
    w0t2, a2p, ind, g0b0, g1b1 = _prep_weights(W0, W1, gamma0, beta0, gamma1, beta1)

    if "nc" not in _CACHE:
        _CACHE["nc"] = _build()
    nc = _CACHE["nc"]

    in_maps = []
    for core in range(NCORES):
        n0 = core * N_LOC
        in_maps.append({
            "y": np.ascontiguousarray(y[:, :, n0:n0 + N_LOC, :]),
            "x": np.ascontiguousarray(x[:, :, n0:n0 + N_LOC, :]),
            "w0t2": w0t2, "a2": a2p, "ind": ind,
            "g0b0": g0b0, "g1b1": g1b1,
        })
    res = run_bass_kernel_spmd(nc, in_maps, list(range(NCORES)), trace=TRACE)
    LAST_EXEC_NS = res.exec_time_ns
    out = np.concatenate([r["out"] for r in res.results], axis=2)
    return out


# revision 13
# speedup vs baseline: 2.1155x; 1.0547x over previous
"""AGConv Trainium2 kernel: 8-core SPMD over the num_points axis.

Reference computation (per full tensors):
  h   = einsum('of,bfnk->bonk', W0, y)             # conv0
  h   = leaky(bn0(h))                              # BN training mode + LeakyReLU(0.2)
  w   = einsum('po,bonk->bpnk', W1, h)             # conv1 -> per-edge [C_out, C_in] mats
  out = einsum('bnkoi,bink->bonk', w_reshaped, x)  # per-edge tiny matmul
  out = leaky(bn1(out))

Key algebra used here: the per-edge tiny matmul folds into one dense matmul,
  out[o,p] = sum_{i,c} A2[i*64+c, o] * Z[i*64+c, p]
with A2[i*64+c, o] = W1[o*6+i, c] and Z[i*64+c, p] = x[i,p] * hhat[c,p].
Z is built from hhat duplicated across partition halves (via a doubled W0^T
stationary) times x rows broadcast across 64 partitions (via tiny indicator
matmuls). BN stats are computed with bn_stats/bn_aggr and all-reduced across
the 8 cores (sum / sum-of-squares payloads).

Sharding: num_points N=2048 is split 8 ways (256 points per core); weights
and BN params are replicated; output gathered by concatenation over N.
"""

import numpy as np

import concourse.bass as bass
import concourse.bacc as bacc
import concourse.mybir as mybir
import concourse.tile as tile
from concourse.bass_utils import run_bass_kernel_spmd

EPS = 1e-5
SLOPE = 0.2

B, C_IN, N, K = 4, 6, 2048, 40
C_FEAT, C_OUT = 12, 64
NCORES = 8
N_LOC = N // NCORES          # 256 points per core
M_B = N_LOC * K              # 10240 positions per batch per core
M = B * M_B                  # 40960 positions per core
FCH = 512                    # free-dim chunk (1 PSUM bank of f32)
CH_PER_B = M_B // FCH        # 20 chunks per batch
NCH = B * CH_PER_B           # 80 chunks per core
HB = 2                       # half-batches per batch for staging loads
CH_PER_HB = CH_PER_B // HB   # 10
M_HB = M_B // HB             # 5120

M_TOT = float(NCORES * M)            # 327680 positions globally
M1_HALF = float(NCORES * M // 2)     # per-partition sample count for stats1

F32 = mybir.dt.float32
BF16 = mybir.dt.bfloat16

TRACE = False
LAST_EXEC_NS = None
_CACHE = {}


def _build():
    nc = bacc.Bacc(
        "TRN2", target_bir_lowering=False, debug=False, num_devices=NCORES,
    )

    y_ext = nc.declare_dram_parameter("y", [B, C_FEAT, N_LOC, K], F32, isOutput=False)
    x_ext = nc.declare_dram_parameter("x", [B, C_IN, N_LOC, K], F32, isOutput=False)
    w0t2_ext = nc.declare_dram_parameter("w0t2", [C_FEAT, 128], F32, isOutput=False)
    a2_ext = nc.declare_dram_parameter("a2", [128, 192], F32, isOutput=False)
    ind_ext = nc.declare_dram_parameter("ind", [C_IN, 384], F32, isOutput=False)
    g0_ext = nc.declare_dram_parameter("g0b0", [128, 2], F32, isOutput=False)
    g1_ext = nc.declare_dram_parameter("g1b1", [128, 2], F32, isOutput=False)
    out_ext = nc.declare_dram_parameter("out", [B, C_OUT, N_LOC, K], F32, isOutput=True)

    y_v = y_ext.ap().rearrange("b c n k -> b c (n k)")
    x_v = x_ext.ap().rearrange("b c n k -> b c (n k)")
    out_v = out_ext.ap().rearrange("b c n k -> b c (n k)")

    with tile.TileContext(nc) as tc:
        with (
            tc.tile_pool(name="const", bufs=1) as cpool,
            tc.tile_pool(name="big", bufs=1) as bpool,
            tc.tile_pool(name="yld", bufs=2) as ypool,
            tc.tile_pool(name="xld", bufs=2) as xpool,
            tc.tile_pool(name="work", bufs=3) as wpool,
            tc.tile_pool(name="stat", bufs=1) as spool,
            tc.tile_pool(name="pa", bufs=2, space="PSUM") as pa_pool,
            tc.tile_pool(name="px", bufs=1, space="PSUM") as px_pool,
            tc.tile_pool(name="pc", bufs=2, space="PSUM") as pc_pool,
            tc.tile_pool(name="dram", bufs=1, space="DRAM") as dpool,
        ):
            # ---- constants ----
            w0t2 = cpool.tile([C_FEAT, 128], BF16, tag="w0t2")
            a2w = cpool.tile([128, 192], BF16, tag="a2w")
            ind = cpool.tile([C_IN, 384], BF16, tag="ind")
            g0b0 = cpool.tile([128, 2], F32, tag="g0b0")
            g1b1 = cpool.tile([128, 2], F32, tag="g1b1")
            nc.gpsimd.dma_start(out=w0t2[:], in_=w0t2_ext.ap())
            nc.gpsimd.dma_start(out=a2w[:], in_=a2_ext.ap())
            nc.gpsimd.dma_start(out=ind[:], in_=ind_ext.ap())
            nc.sync.dma_start(out=g0b0[:], in_=g0_ext.ap())
            nc.sync.dma_start(out=g1b1[:], in_=g1_ext.ap())

            # ---- persistent buffers ----
            op_all = bpool.tile([128, M // 2], BF16, tag="op_all")  # 40KB/part
            st0 = spool.tile([128, 6 * NCH], F32, tag="st0")
            st1 = spool.tile([128, 6 * (NCH // 2)], F32, tag="st1")

            # =========== Phase A: h = W0 @ y (doubled), stats0 ===========
            for b in range(B):
                y_hb = ypool.tile([C_FEAT, M_B], BF16, tag="y_hb")
                nc.gpsimd.dma_start(out=y_hb[:], in_=y_v[b, :, :])
                for t in range(CH_PER_B):
                        idx = b * CH_PER_B + t
                        pA = pa_pool.tile([128, FCH], F32, tag="pA")
                        nc.tensor.matmul(
                            pA[:], w0t2[:], y_hb[:, t * FCH:(t + 1) * FCH],
                            start=True, stop=True,
                        )
                        nc.vector.bn_stats(st0[:, idx * 6:(idx + 1) * 6], pA[:])

            # =========== All-reduce stats0 -> a0/b0 scale-bias ===========
            agg0 = spool.tile([128, 2], F32, tag="agg0")
            pay0 = spool.tile([128, 2], F32, tag="pay0")
            nc.vector.bn_aggr(agg0[:], st0[:])
            # payload: [sum, sumsq] = [mean*M, (var+mean^2)*M]
            m2a = spool.tile([128, 1], F32, tag="m2a")
            nc.vector.tensor_mul(m2a[:], agg0[:, 0:1], agg0[:, 0:1])
            nc.vector.tensor_add(pay0[:, 1:2], agg0[:, 1:2], m2a[:])
            nc.vector.tensor_copy(pay0[:, 0:1], agg0[:, 0:1])
            nc.vector.tensor_scalar_mul(pay0[:], pay0[:], float(M))

            cc0_in = dpool.tile([128, 2], F32, tag="cc0i")
            cc0_out = dpool.tile([128, 2], F32, tag="cc0o")
            nc.sync.dma_start(out=cc0_in[:], in_=pay0[:])
            nc.gpsimd.collective_compute(
                "AllReduce", mybir.AluOpType.add,
                replica_groups=[list(range(NCORES))],
                ins=[cc0_in.opt()], outs=[cc0_out.opt()],
            )
            gs0 = spool.tile([128, 2], F32, tag="gs0")
            nc.sync.dma_start(out=gs0[:], in_=cc0_out[:])

            ab0 = spool.tile([128, 2], F32, tag="ab0")  # col0=a, col1=b
            mean0 = spool.tile([128, 1], F32, tag="mean0")
            var0 = spool.tile([128, 1], F32, tag="var0")
            sd0 = spool.tile([128, 1], F32, tag="sd0")
            t0 = spool.tile([128, 1], F32, tag="t0")
            nc.vector.tensor_scalar_mul(gs0[:], gs0[:], 1.0 / M_TOT)
            nc.vector.tensor_copy(mean0[:], gs0[:, 0:1])
            nc.vector.tensor_mul(t0[:], mean0[:], mean0[:])
            nc.vector.tensor_sub(var0[:], gs0[:, 1:2], t0[:])
            nc.vector.tensor_scalar_add(var0[:], var0[:], EPS)
            nc.scalar.activation(sd0[:], var0[:], mybir.ActivationFunctionType.Sqrt)
            nc.vector.reciprocal(sd0[:], sd0[:])
            nc.vector.tensor_mul(ab0[:, 0:1], sd0[:], g0b0[:, 0:1])
            nc.vector.tensor_mul(t0[:], ab0[:, 0:1], mean0[:])
            nc.vector.tensor_sub(ab0[:, 1:2], g0b0[:, 1:2], t0[:])

            # =========== Phase B: hh=lrelu(a*h+b); Z; out_pre; stats1 ====
            for b in range(B):
                x_hb = xpool.tile([C_IN, M_B], BF16, tag="x_hb")
                y_hb = ypool.tile([C_FEAT, M_B], BF16, tag="y_hb")
                nc.gpsimd.dma_start(out=x_hb[:], in_=x_v[b, :, :])
                nc.gpsimd.dma_start(out=y_hb[:], in_=y_v[b, :, :])
                for t in range(CH_PER_B):
                        idx = b * CH_PER_B + t
                        pB = pa_pool.tile([128, FCH], F32, tag="pA")
                        nc.tensor.matmul(
                            pB[:], w0t2[:], y_hb[:, t * FCH:(t + 1) * FCH],
                            start=True, stop=True,
                        )
                        hh2 = wpool.tile([128, FCH], BF16, tag="hh2")
                        nc.scalar.activation(
                            hh2[:], pB[:],
                            mybir.ActivationFunctionType.Prelu,
                            bias=ab0[:, 1:2], scale=ab0[:, 0:1], alpha=SLOPE,
                        )
                        if idx % 2 == 0:
                            pC = pc_pool.tile([128, FCH], F32, tag="pC")
                        half = pC[0:64, :] if idx % 2 == 0 else pC[64:128, :]
                        for j in range(3):
                            pX = px_pool.tile([128, FCH], F32, tag=f"px{j}")
                            nc.tensor.matmul(
                                pX[:], ind[:, 128 * j:128 * (j + 1)],
                                x_hb[:, t * FCH:(t + 1) * FCH],
                                start=True, stop=True,
                            )
                            Z = wpool.tile([128, FCH], BF16, tag=f"z{j}")
                            nc.vector.tensor_mul(Z[:], hh2[:], pX[:])
                            nc.tensor.matmul(
                                half, a2w[:, 64 * j:64 * (j + 1)], Z[:],
                                start=(j == 0), stop=(j == 2),
                            )
                        if idx % 2 == 1:
                            pr = idx // 2
                            nc.vector.bn_stats(st1[:, pr * 6:(pr + 1) * 6], pC[:])
                            nc.vector.tensor_copy(
                                op_all[:, pr * FCH:(pr + 1) * FCH], pC[:])

            # =========== All-reduce stats1 -> a1/b1 ===========
            agg1 = spool.tile([128, 2], F32, tag="agg1")
            pay1 = spool.tile([128, 2], F32, tag="pay1")
            nc.vector.bn_aggr(agg1[:], st1[:])
            m2b = spool.tile([128, 1], F32, tag="m2b")
            nc.vector.tensor_mul(m2b[:], agg1[:, 0:1], agg1[:, 0:1])
            nc.vector.tensor_add(pay1[:, 1:2], agg1[:, 1:2], m2b[:])
            nc.vector.tensor_copy(pay1[:, 0:1], agg1[:, 0:1])
            nc.vector.tensor_scalar_mul(pay1[:], pay1[:], float(M // 2))

            cc1_in = dpool.tile([128, 2], F32, tag="cc1i")
            cc1_out = dpool.tile([128, 2], F32, tag="cc1o")
            nc.sync.dma_start(out=cc1_in[:], in_=pay1[:])
            nc.gpsimd.collective_compute(
                "AllReduce", mybir.AluOpType.add,
                replica_groups=[list(range(NCORES))],
                ins=[cc1_in.opt()], outs=[cc1_out.opt()],
            )
            gs1 = spool.tile([128, 2], F32, tag="gs1")
            nc.sync.dma_start(out=gs1[:], in_=cc1_out[:])

            # combine partition halves (each half saw different chunks)
            tmph = spool.tile([64, 2], F32, tag="tmph")
            tot1 = spool.tile([64, 2], F32, tag="tot1")
            nc.sync.dma_start(out=tmph[:], in_=gs1[64:128, :])
            nc.vector.tensor_add(tot1[:], gs1[0:64, :], tmph[:])

            ab1 = spool.tile([128, 2], F32, tag="ab1")
            mean1 = spool.tile([64, 1], F32, tag="mean1")
            var1 = spool.tile([64, 1], F32, tag="var1")
            sd1 = spool.tile([64, 1], F32, tag="sd1")
            t1 = spool.tile([64, 1], F32, tag="t1")
            nc.vector.tensor_scalar_mul(tot1[:], tot1[:], 1.0 / M_TOT)
            nc.vector.tensor_copy(mean1[:], tot1[:, 0:1])
            nc.vector.tensor_mul(t1[:], mean1[:], mean1[:])
            nc.vector.tensor_sub(var1[:], tot1[:, 1:2], t1[:])
            nc.vector.tensor_scalar_add(var1[:], var1[:], EPS)
            nc.scalar.activation(sd1[:], var1[:], mybir.ActivationFunctionType.Sqrt)
            nc.vector.reciprocal(sd1[:], sd1[:])
            nc.vector.tensor_mul(ab1[0:64, 0:1], sd1[:], g1b1[0:64, 0:1])
            nc.vector.tensor_mul(t1[:], ab1[0:64, 0:1], mean1[:])
            nc.vector.tensor_sub(ab1[0:64, 1:2], g1b1[0:64, 1:2], t1[:])
            nc.sync.dma_start(out=ab1[64:128, :], in_=ab1[0:64, :])

            # =========== Phase C: out = lrelu(a1*out_pre+b1) -> DRAM =====
            # out partition layout is (two, c): chunk parity in the top
            # 64 partitions; one big strided DMA per (batch, half) group.
            out_q = out_v.rearrange(
                "b c (hb p two w) -> b c hb two p w", hb=HB, two=2, w=FCH)
            GW = 5 * FCH  # columns per group
            for g in range(NCH // 10):
                gb, ghb = g // HB, g % HB
                outst = wpool.tile([128, GW], F32, tag="outf")
                nc.scalar.activation(
                    outst[:], op_all[:, g * GW:(g + 1) * GW],
                    mybir.ActivationFunctionType.Prelu,
                    bias=ab1[:, 1:2], scale=ab1[:, 0:1], alpha=SLOPE,
                )
                for two in range(2):
                    nc.sync.dma_start(
                        out=out_q[gb, :, ghb, two, :, :],
                        in_=outst[64 * two:64 * (two + 1), :].rearrange(
                            "q (p w) -> q p w", w=FCH),
                    )
    nc.finalize()
    return nc


def _prep_weights(W0, W1, gamma0, beta0, gamma1, beta1):
    W0 = np.asarray(W0, np.float32)
    W1 = np.asarray(W1, np.float32)
    w0t2 = np.concatenate([W0.T, W0.T], axis=1)  # [12, 128]
    # A2[i*64+c, o] = W1[o*6+i, c]
    A2 = W1.reshape(C_OUT, C_IN, C_OUT).transpose(1, 2, 0).reshape(384, C_OUT)
    a2p = np.concatenate([A2[0:128], A2[128:256], A2[256:384]], axis=1)  # [128,192]
    ind = np.zeros((C_IN, 384), np.float32)
    for j in range(3):
        for p in range(128):
            ind[2 * j + p // 64, 128 * j + p] = 1.0
    g0b0 = np.stack([np.tile(np.asarray(gamma0, np.float32), 2),
                     np.tile(np.asarray(beta0, np.float32), 2)], axis=1)
    g1b1 = np.stack([np.tile(np.asarray(gamma1, np.float32), 2),
                     np.tile(np.asarray(beta1, np.float32), 2)], axis=1)
    return (np.ascontiguousarray(w0t2, np.float32),
            np.ascontiguousarray(a2p, np.float32),
            np.ascontiguousarray(ind, np.float32),
            np.ascontiguousarray(g0b0, np.float32),
            np.ascontiguousarray(g1b1, np.float32))


def kernel(x, y, W0, gamma0, beta0, W1, gamma1, beta1):
    global LAST_EXEC_NS
    x = np.asarray(x, np.float32)
    y = np.asarray(y, np.float32)
    w0t2, a2p, ind, g0b0, g1b1 = _prep_weights(W0, W1, gamma0, beta0, gamma1, beta1)

    if "nc" not in _CACHE:
        _CACHE["nc"] = _build()
    nc = _CACHE["nc"]

    in_maps = []
    for core in range(NCORES):
        n0 = core * N_LOC
        in_maps.append({
            "y": np.ascontiguousarray(y[:, :, n0:n0 + N_LOC, :]),
            "x": np.ascontiguousarray(x[:, :, n0:n0 + N_LOC, :]),
            "w0t2": w0t2, "a2": a2p, "ind": ind,
            "g0b0": g0b0, "g1b1": g1b1,
        })
    res = run_bass_kernel_spmd(nc, in_maps, list(range(NCORES)), trace=TRACE)
    LAST_EXEC_NS = res.exec_time_ns
    out = np.concatenate([r["out"] for r in res.results], axis=2)
    return out
